# revision 1
# baseline (speedup 1.0000x reference)
"""Trainium2 Bass kernel for nn_NewCNNEncoder (dense CNN encoder over one-hot boards).

Strategy (pure data parallel over 8 NeuronCores, 8192 samples each):
  - The input x [B, 25] (values 0..16) is one-hot encoded ON CHIP via
    broadcast-matmul + is_equal compare, in three layouts matched to the
    three depthwise-conv branches (full / horizontal / vertical).
  - All convolutions are expressed as dense matmuls with activations kept
    in [features-on-partitions, batch-free] layout; the final conv_out
    layer flips to [batch-on-partitions, features-free] so the output DMA
    is contiguous.
  - Matmuls run in float32r (full-rate fp32 mode) except the first-layer
    broadcast and the last layer, which run in bf16.
  - leaky_relu(+bias) epilogues are single ScalarE activation ops reading
    PSUM directly.
"""

import sys

sys.path.insert(0, "/opt/trn_rl_repo")

import numpy as np
import ml_dtypes

import concourse.mybir as mybir
import concourse.tile as tile
from concourse import bacc
from concourse.bass_utils import run_bass_kernel_spmd

NCORES = 8
B_FULL = 65536
BC = B_FULL // NCORES  # 8192 per core
NT = 512               # batch tile (samples per pipeline tile)
NTILES = BC // NT      # 16

NC_ = 25   # cells (5x5 board)
NCL = 17   # classes
MULT = 16
OC = 64
OF = 1600
SLOPE = 0.01

F32 = mybir.dt.float32
F32R = mybir.dt.float32r
BF16 = mybir.dt.bfloat16
BF16NP = ml_dtypes.bfloat16
LRELU = mybir.ActivationFunctionType.Lrelu
EQ = mybir.AluOpType.is_equal

# one-hot row chunking, 425 rows (p = 25c + l) zero-padded to 512
FULL_CH = [(0, 128), (128, 128), (256, 128), (384, 128)]
# per-r (and per-j) L1 output col chunks, 272 (16c + m) zero-padded to 384
H_CH = [(0, 128), (128, 128), (256, 128)]
# L2-full output chunks over 320
F2_CH = [(0, 128), (128, 128), (256, 64)]
# act2 (cat) K-chunk sizes: 7x128 + 65 (last = vert j4 (64) + ones row)
A2_SIZES = [128] * 7 + [65]
# where each L2 output block lands in the A2 tiles: branch -> (tile, part_off)
H_DST = {0: (2, 64), 1: (3, 0), 2: (3, 64), 3: (4, 0), 4: (4, 64)}
V_DST = {0: (5, 0), 1: (5, 64), 2: (6, 0), 3: (6, 64), 4: (7, 0)}


def _fr(c, l):
    """class-aligned padded row/feature index for (class, cell)"""
    return 128 * (c // 5) + 25 * (c % 5) + l


def _build_nc():
    nc = bacc.Bacc("TRN2", target_bir_lowering=False, debug=False)

    # ---- DRAM I/O ----
    d_xt = nc.dram_tensor("xt", [128, BC], BF16, kind="ExternalInput")
    d_sf = nc.dram_tensor("sf", [128, 512], BF16, kind="ExternalInput")
    d_sh = nc.dram_tensor("sh", [128, 425], BF16, kind="ExternalInput")
    d_sv = nc.dram_tensor("sv", [128, 425], BF16, kind="ExternalInput")
    d_clsf = nc.dram_tensor("clsf", [128, 4], F32, kind="ExternalInput")
    d_clsh = nc.dram_tensor("clsh", [85, 1], F32, kind="ExternalInput")
    d_a1f = nc.dram_tensor("a1f", [512, 512], F32R, kind="ExternalInput")
    d_a1h = nc.dram_tensor("a1h", [85, 384], F32R, kind="ExternalInput")
    d_a1v = nc.dram_tensor("a1v", [85, 384], F32R, kind="ExternalInput")
    d_w2f = nc.dram_tensor("w2f", [512, 320], F32R, kind="ExternalInput")
    d_w2h = nc.dram_tensor("w2h", [384, 64], F32R, kind="ExternalInput")
    d_w2v = nc.dram_tensor("w2v", [384, 64], F32R, kind="ExternalInput")
    d_w3 = nc.dram_tensor("w3", [961, OF], BF16, kind="ExternalInput")
    d_b1f = nc.dram_tensor("b1f", [128, 4], F32, kind="ExternalInput")
    d_b1h = nc.dram_tensor("b1h", [128, 3], F32, kind="ExternalInput")
    d_b1v = nc.dram_tensor("b1v", [128, 3], F32, kind="ExternalInput")
    d_b2f = nc.dram_tensor("b2f", [128, 3], F32, kind="ExternalInput")
    d_b2h = nc.dram_tensor("b2h", [64, 1], F32, kind="ExternalInput")
    d_b2v = nc.dram_tensor("b2v", [64, 1], F32, kind="ExternalInput")
    d_y = nc.dram_tensor("y", [BC, OF], F32, kind="ExternalOutput")

    with tile.TileContext(nc) as tc:
        with (
            tc.tile_pool(name="const", bufs=1) as cp,
            tc.tile_pool(name="work", bufs=2) as wp,
            tc.tile_pool(name="oh", bufs=3) as ohp,
            tc.tile_pool(name="outp", bufs=3) as op_,
            tc.tile_pool(name="ps_s", bufs=4, space="PSUM") as pp,
            tc.tile_pool(name="ps_l3", bufs=2, space="PSUM") as pp3,
        ):
            # ---- load constants/weights into SBUF ----
            xt = cp.tile([128, BC], BF16, tag="xt")
            for t_i in range(NTILES):
                nc.sync.dma_start(xt[:, t_i * NT:(t_i + 1) * NT],
                                  d_xt[:, t_i * NT:(t_i + 1) * NT])
            sf = cp.tile([128, 512], BF16, tag="sf")
            nc.sync.dma_start(sf[:], d_sf[:])
            sh = cp.tile([128, 425], BF16, tag="sh")
            nc.sync.dma_start(sh[:], d_sh[:])
            sv = cp.tile([128, 425], BF16, tag="sv")
            nc.sync.dma_start(sv[:], d_sv[:])
            clsf = cp.tile([128, 4], F32, tag="clsf")
            nc.sync.dma_start(clsf[:], d_clsf[:])
            clsh = cp.tile([85, 1], F32, tag="clsh")
            nc.sync.dma_start(clsh[:], d_clsh[:])

            a1f = []
            for kc, (k0, kp) in enumerate(FULL_CH):
                t = cp.tile([kp, 512], F32R, tag=f"a1f_{kc}")
                nc.sync.dma_start(t[:], d_a1f[k0:k0 + kp, :])
                a1f.append(t)
            a1h = cp.tile([85, 384], F32R, tag="a1h")
            nc.sync.dma_start(a1h[:], d_a1h[:])
            a1v = cp.tile([85, 384], F32R, tag="a1v")
            nc.sync.dma_start(a1v[:], d_a1v[:])

            w2f = []
            for kc, (k0, kp) in enumerate(FULL_CH):
                t = cp.tile([kp, 320], F32R, tag=f"w2f_{kc}")
                nc.sync.dma_start(t[:], d_w2f[k0:k0 + kp, :])
                w2f.append(t)
            w2h = []
            w2v = []
            for kc, (k0, kp) in enumerate(H_CH):
                t = cp.tile([kp, 64], F32R, tag=f"w2h_{kc}")
                nc.sync.dma_start(t[:], d_w2h[k0:k0 + kp, :])
                w2h.append(t)
                t = cp.tile([kp, 64], F32R, tag=f"w2v_{kc}")
                nc.sync.dma_start(t[:], d_w2v[k0:k0 + kp, :])
                w2v.append(t)
            w3 = []
            r0 = 0
            for i, sz in enumerate(A2_SIZES):
                t = cp.tile([sz, OF], BF16, tag=f"w3_{i}")
                nc.sync.dma_start(t[:], d_w3[r0:r0 + sz, :])
                w3.append(t)
                r0 += sz

            b1f = cp.tile([128, 4], F32, tag="b1f")
            nc.sync.dma_start(b1f[:], d_b1f[:])
            b1h = cp.tile([128, 3], F32, tag="b1h")
            nc.sync.dma_start(b1h[:], d_b1h[:])
            b1v = cp.tile([128, 3], F32, tag="b1v")
            nc.sync.dma_start(b1v[:], d_b1v[:])
            b2f = cp.tile([128, 3], F32, tag="b2f")
            nc.sync.dma_start(b2f[:], d_b2f[:])
            b2h = cp.tile([64, 1], F32, tag="b2h")
            nc.sync.dma_start(b2h[:], d_b2h[:])
            b2v = cp.tile([64, 1], F32, tag="b2v")
            nc.sync.dma_start(b2v[:], d_b2v[:])

            # ---- batch-tile pipeline ----
            for t_i in range(NTILES):
                n0 = t_i * NT
                xs = xt[:, n0:n0 + NT]

                A2 = [wp.tile([A2_SIZES[i], NT], BF16, tag=f"a2_{i}",
                              name=f"a2_{i}_{t_i}")
                      for i in range(8)]

                # ===== full branch =====
                ohf = []
                for kc, (k0, kp) in enumerate(FULL_CH):
                    ps = pp.tile([kp, NT], F32, tag="ps_s")
                    nc.tensor.matmul(ps[:], sf[:, k0:k0 + kp], xs,
                                     start=True, stop=True)
                    oht = ohp.tile([kp, NT], F32R, tag=f"ohf{kc}")
                    nc.vector.tensor_scalar(oht[:], ps[:],
                                            clsf[0:kp, kc:kc + 1], None,
                                            op0=EQ)
                    ohf.append(oht)

                act1f = []
                for mc, (m0, mp) in enumerate(FULL_CH):
                    ps = pp.tile([mp, NT], F32, tag="ps_s")
                    nc.tensor.matmul(ps[:], a1f[mc][:, m0:m0 + mp],
                                     ohf[mc][:], start=True, stop=True)
                    a = wp.tile([mp, NT], F32R, tag=f"act1f{mc}")
                    nc.scalar.activation(a[:], ps[:], LRELU,
                                         bias=b1f[0:mp, mc:mc + 1],
                                         alpha=SLOPE)
                    act1f.append(a)

                for mc2, (m0, mp) in enumerate(F2_CH):
                    ps = pp.tile([mp, NT], F32, tag="ps_s")
                    for i in range(4):
                        nc.tensor.matmul(ps[:], w2f[i][:, m0:m0 + mp],
                                         act1f[i][:],
                                         start=(i == 0), stop=(i == 3))
                    if mc2 < 2:
                        dst = A2[mc2][0:128, :]
                    else:
                        dst = A2[2][0:64, :]
                    nc.scalar.activation(dst, ps[:], LRELU,
                                         bias=b2f[0:mp, mc2:mc2 + 1],
                                         alpha=SLOPE)

                # ===== hori / vert branches: stage-offset software pipeline =====
                BR = {
                    "h": (sh, a1h, w2h, b1h, b2h, H_DST),
                    "v": (sv, a1v, w2v, b1v, b2v, V_DST),
                }
                pairs = [("h", r) for r in range(5)] + \
                        [("v", r) for r in range(5)]
                oh_l, a1_l = {}, {}

                def _gen(br, r):
                    s_mat = BR[br][0]
                    ps = pp.tile([85, NT], F32, tag="ps_s",
                                 name=f"psb_{br}{r}_{t_i}")
                    nc.tensor.matmul(ps[:], s_mat[:, 85 * r:85 * r + 85],
                                     xs, start=True, stop=True)
                    ohr = ohp.tile([85, NT], F32R, tag=f"oh{br}",
                                   name=f"oh{br}{r}_{t_i}")
                    nc.vector.tensor_scalar(ohr[:], ps[:], clsh[:, 0:1],
                                            None, op0=EQ)
                    oh_l[(br, r)] = ohr

                def _l1(br, r):
                    a1_mat, b1_t = BR[br][1], BR[br][3]
                    ohr = oh_l.pop((br, r))
                    ts = []
                    for mc, (m0, mp) in enumerate(H_CH):
                        ps1 = pp.tile([mp, NT], F32, tag="ps_s",
                                      name=f"ps1_{br}{r}{mc}_{t_i}")
                        nc.tensor.matmul(ps1[:], a1_mat[:, m0:m0 + mp],
                                         ohr[:], start=True, stop=True)
                        a = wp.tile([mp, NT], F32R, tag=f"act1{br}{mc}",
                                    name=f"act1{br}{r}{mc}_{t_i}")
                        nc.scalar.activation(a[:], ps1[:], LRELU,
                                             bias=b1_t[0:mp, mc:mc + 1],
                                             alpha=SLOPE)
                        ts.append(a)
                    a1_l[(br, r)] = ts

                def _l2(br, r):
                    w2_t, b2_t, dst_map = BR[br][2], BR[br][4], BR[br][5]
                    a1_t = a1_l.pop((br, r))
                    ps2 = pp.tile([64, NT], F32, tag="ps_s",
                                  name=f"ps2_{br}{r}_{t_i}")
                    for i, (m0, mp) in enumerate(H_CH):
                        nc.tensor.matmul(ps2[:], w2_t[i][:, 0:64],
                                         a1_t[i][:],
                                         start=(i == 0), stop=(i == 2))
                    ti, off = dst_map[r]
                    nc.scalar.activation(A2[ti][off:off + 64, :], ps2[:],
                                         LRELU, bias=b2_t[0:64, 0:1],
                                         alpha=SLOPE)

                for idx in range(len(pairs) + 2):
                    if idx < len(pairs):
                        _gen(*pairs[idx])
                    if 0 <= idx - 1 < len(pairs):
                        _l1(*pairs[idx - 1])
                    if 0 <= idx - 2 < len(pairs):
                        _l2(*pairs[idx - 2])

                # ones row for the bias of the output layer
                nc.vector.memset(A2[7][64:65, :], 1.0)

                # ===== output layer (batch on partitions) =====
                for q in range(4):
                    b0 = q * 128
                    o = op_.tile([128, OF], F32, tag="outt")
                    for half in range(2):
                        c0 = half * 800
                        ps3 = pp3.tile([128, 800], F32, tag="ps_l3",
                                       name=f"ps3_{q}{half}_{t_i}")
                        for i in range(8):
                            lh = A2[i][:, b0:b0 + 128]
                            st, sp_ = (i == 0), (i == 7)
                            nc.tensor.matmul(ps3[:, 0:512], lh,
                                             w3[i][:, c0:c0 + 512],
                                             start=st, stop=sp_)
                            nc.tensor.matmul(ps3[:, 512:800], lh,
                                             w3[i][:, c0 + 512:c0 + 800],
                                             start=st, stop=sp_)
                        nc.scalar.activation(o[:, c0:c0 + 800], ps3[:],
                                             LRELU, alpha=SLOPE)
                    nc.sync.dma_start(d_y[n0 + b0:n0 + b0 + 128, :], o[:])

    nc.compile()
    return nc


_NC_CACHE = None


def _get_nc():
    global _NC_CACHE
    if _NC_CACHE is None:
        _NC_CACHE = _build_nc()
    return _NC_CACHE


def _prep_weights(inputs):
    W_df = np.asarray(inputs["W_df"], dtype=np.float32)
    b_df = np.asarray(inputs["b_df"], dtype=np.float32)
    W_pf = np.asarray(inputs["W_pf"], dtype=np.float32)
    b_pf = np.asarray(inputs["b_pf"], dtype=np.float32)
    W_dh = np.asarray(inputs["W_dh"], dtype=np.float32)
    b_dh = np.asarray(inputs["b_dh"], dtype=np.float32)
    W_ph = np.asarray(inputs["W_ph"], dtype=np.float32)
    b_ph = np.asarray(inputs["b_ph"], dtype=np.float32)
    W_dv = np.asarray(inputs["W_dv"], dtype=np.float32)
    b_dv = np.asarray(inputs["b_dv"], dtype=np.float32)
    W_pv = np.asarray(inputs["W_pv"], dtype=np.float32)
    b_pv = np.asarray(inputs["b_pv"], dtype=np.float32)
    W_out = np.asarray(inputs["W_out"], dtype=np.float32)
    b_out = np.asarray(inputs["b_out"], dtype=np.float32)

    cc = np.arange(NCL)
    ll = np.arange(NC_)

    A_full = np.zeros((512, 512), np.float32)
    for c in range(NCL):
        r0, c0 = _fr(c, 0), _fr(c, 0)
        # block [l, m] = W_df[c, m, l]
        A_full[r0:r0 + 25, c0:c0 + 25] = W_df[c].T
    A_h = np.zeros((85, 384), np.float32)
    A_v = np.zeros((85, 384), np.float32)
    for c in range(NCL):
        A_h[5 * c:5 * c + 5, 16 * c:16 * c + 16] = W_dh[c].T  # [j, m]
        A_v[5 * c:5 * c + 5, 16 * c:16 * c + 16] = W_dv[c].T  # [r, m]

    # selection (broadcast) matrices, bf16-exact 0/1 (K padded 25 -> 128)
    sf = np.zeros((128, 512), BF16NP)
    for c in range(NCL):
        for l in range(NC_):
            sf[l, _fr(c, l)] = 1
    # sh: col 85*r + 5*c + j -> row l = 5*r + j
    sh = np.zeros((128, 425), BF16NP)
    # sv: col 85*j + 5*c + r -> row l = 5*r + j
    sv = np.zeros((128, 425), BF16NP)
    for c in range(NCL):
        for r in range(5):
            for j in range(5):
                sh[5 * r + j, 85 * r + 5 * c + j] = 1
                sv[5 * r + j, 85 * j + 5 * c + r] = 1

    # class constant per one-hot row; -1 on padding rows (matches nothing)
    clsf = np.full((128, 4), -1.0, np.float32)
    for kc in range(4):
        for c in range(5 * kc, min(5 * kc + 5, NCL)):
            p0 = 25 * (c % 5)
            clsf[p0:p0 + 25, kc] = float(c)
    clsh = (np.arange(85) // 5).astype(np.float32)[:, None]

    # output-layer weights, rows reordered to the act2 chunk layout
    W3re = np.zeros((961, OF), np.float32)
    W3re[0:320] = W_out[:, :, 0:5].transpose(1, 2, 0).reshape(320, OF)
    W3re[320:640] = W_out[:, :, 5:10].transpose(2, 1, 0).reshape(320, OF)
    W3re[640:960] = W_out[:, :, 10:15].transpose(2, 1, 0).reshape(320, OF)
    W3re[960] = b_out

    b1f = np.zeros((128, 4), np.float32)
    for mc in range(4):
        for c in range(5 * mc, min(5 * mc + 5, NCL)):
            p0 = 25 * (c % 5)
            b1f[p0:p0 + 25, mc] = b_df[25 * c:25 * c + 25]
    b1h = np.zeros((128, 3), np.float32)
    b1v = np.zeros((128, 3), np.float32)
    for mc, (m0, mp) in enumerate(H_CH):
        valid = max(0, min(mp, 272 - m0))
        b1h[0:valid, mc] = b_dh[m0:m0 + valid]
        b1v[0:valid, mc] = b_dv[m0:m0 + valid]
    b2f = np.zeros((128, 3), np.float32)
    for mc, (m0, mp) in enumerate(F2_CH):
        b2f[0:mp, mc] = b_pf[m0:m0 + mp]

    w2f_p = np.zeros((512, 320), np.float32)
    for c in range(NCL):
        r0 = _fr(c, 0)
        w2f_p[r0:r0 + 25] = W_pf.T[25 * c:25 * c + 25]
    w2h_p = np.zeros((384, 64), np.float32)
    w2h_p[0:272] = W_ph.T
    w2v_p = np.zeros((384, 64), np.float32)
    w2v_p[0:272] = W_pv.T

    return {
        "sf": sf, "sh": sh, "sv": sv,
        "clsf": clsf, "clsh": clsh,
        "a1f": A_full, "a1h": A_h, "a1v": A_v,
        "w2f": w2f_p, "w2h": w2h_p, "w2v": w2v_p,
        "w3": W3re.astype(BF16NP),
        "b1f": b1f, "b1h": b1h, "b1v": b1v,
        "b2f": b2f,
        "b2h": b_ph[:, None].copy(),
        "b2v": b_pv[:, None].copy(),
    }


def kernel(**inputs) -> np.ndarray:
    x = np.asarray(inputs["x"]).astype(np.int32)
    assert x.shape == (B_FULL, NC_), x.shape

    shared = _prep_weights(inputs)
    nc = _get_nc()

    in_maps = []
    for core in range(NCORES):
        xs = x[core * BC:(core + 1) * BC]          # [BC, 25]
        xtc = np.zeros((128, BC), BF16NP)
        xtc[:NC_] = xs.T.astype(BF16NP)
        m = dict(shared)
        m["xt"] = xtc
        in_maps.append(m)

    res = run_bass_kernel_spmd(nc, in_maps, core_ids=list(range(NCORES)))
    global LAST_RESULTS
    LAST_RESULTS = res
    out = np.concatenate([res.results[i]["y"] for i in range(NCORES)], axis=0)
    return out


LAST_RESULTS = None



# revision 2
# speedup vs baseline: 1.0192x; 1.0192x over previous
"""Trainium2 Bass kernel v2 for nn_NewCNNEncoder (dense CNN encoder).

Design (pure data parallel over 8 NeuronCores, 8192 samples each):
  - One-hot encodings are built ON HOST as fp8 (exact 0/1) in the three
    layouts the depthwise branches need, with dual-scale slot pairs
    (1, 1/16) so L1 runs as fp8 DoubleRow matmuls with hi/lo split
    weights at ~full precision: W ~= q8(W) + q8(16 dW)/16. This removes
    all on-chip one-hot generation (14 matmuls + 14 vector compares per
    batch tile in v1) and halves the one-hot DMA vs bf16.
  - L2 (pointwise) layers run in bf16; the h and v branches are merged
    into one 640-row K-stack so each r needs 5 matmuls + 5 activations
    instead of 6+6.
  - All biases are folded into matmuls via ones-rows in the one-hot /
    act1 tiles; every activation epilogue is a pure leaky-relu, so some
    can run on the vector engine (2-op mult+max) to unload the scalar
    engine.
  - conv_out (the 961x1600 dense layer, ~60% of PE cycles) stays bf16
    with batch on PSUM partitions so the output DMA is contiguous.
"""

import sys

sys.path.insert(0, "/opt/trn_rl_repo")

import numpy as np
import ml_dtypes

import concourse.mybir as mybir
import concourse.tile as tile
from concourse import bacc
from concourse.bass_utils import run_bass_kernel_spmd

NCORES = 8
B_FULL = 65536
BC = B_FULL // NCORES  # 8192
NT = 512
NTILES = BC // NT      # 16

NCL = 17
MULT = 16
OC = 64
OF = 1600
SLOPE = 0.01

F32 = mybir.dt.float32
BF16 = mybir.dt.bfloat16
F8 = mybir.dt.float8e4
DRM = mybir.MatmulPerfMode.DoubleRow
LRELU = mybir.ActivationFunctionType.Lrelu
F8NP = ml_dtypes.float8_e4m3fn
BF16NP = ml_dtypes.bfloat16

# how many of the L1hv activation epilogues go to the vector engine
L1HV_VEC = 16

ONE8 = np.float32(1.0).astype(F8NP)
SIXT8 = np.float32(0.0625).astype(F8NP)


def _wsplit16(W):
    W = np.asarray(W, np.float32)
    hi = W.astype(F8NP)
    lo = (16.0 * (W - hi.astype(np.float32))).astype(F8NP)
    return hi, lo


def _build_nc():
    nc = bacc.Bacc("TRN2", target_bir_lowering=False, debug=False)

    d_oh = nc.dram_tensor("oh", [19, 128, 2, BC], F8, kind="ExternalInput")
    d_a1f = nc.dram_tensor("a1f", [4, 128, 2, 128], F8, kind="ExternalInput")
    d_a1h = nc.dram_tensor("a1h", [128, 2, 256], F8, kind="ExternalInput")
    d_a1v = nc.dram_tensor("a1v", [128, 2, 256], F8, kind="ExternalInput")
    d_a1t = nc.dram_tensor("a1t", [128, 2, 128], F8, kind="ExternalInput")
    d_w2f = nc.dram_tensor("w2f", [4, 128, 384], BF16, kind="ExternalInput")
    d_w2hv = nc.dram_tensor("w2hv", [5, 128, 128], BF16, kind="ExternalInput")
    d_w3 = nc.dram_tensor("w3", [8, 128, OF], BF16, kind="ExternalInput")
    d_y = nc.dram_tensor("y", [BC, OF], F32, kind="ExternalOutput")

    # L2 output chunk -> A2 chunk index
    L2F_DST = {0: 0, 1: 1, 2: 7}
    L2HV_DST = {r: 2 + r for r in range(5)}
    H_CH = [(0, 128), (128, 128), (256, 64)]

    with tile.TileContext(nc) as tc:
        with (
            tc.tile_pool(name="const", bufs=1) as cp,
            tc.tile_pool(name="ohp", bufs=2) as ohp,
            tc.tile_pool(name="a1fp", bufs=2) as a1fp,
            tc.tile_pool(name="stkp", bufs=2) as stkp,
            tc.tile_pool(name="a2p", bufs=2) as a2p,
            tc.tile_pool(name="vtp", bufs=2) as vtp,
            tc.tile_pool(name="outp", bufs=3) as outp,
            tc.tile_pool(name="ps", bufs=4, space="PSUM") as pp,
            tc.tile_pool(name="ps3", bufs=2, space="PSUM") as pp3,
        ):
            # ---- load weights ----
            a1f = []
            for k in range(4):
                t = cp.tile([128, 2, 128], F8, tag=f"a1f{k}")
                nc.sync.dma_start(t[:], d_a1f[k])
                a1f.append(t)
            a1h = cp.tile([128, 2, 256], F8, tag="a1h")
            nc.sync.dma_start(a1h[:], d_a1h[:])
            a1v = cp.tile([128, 2, 256], F8, tag="a1v")
            nc.sync.dma_start(a1v[:], d_a1v[:])
            a1t = cp.tile([128, 2, 128], F8, tag="a1t")
            nc.sync.dma_start(a1t[:], d_a1t[:])
            w2f = []
            for k in range(4):
                t = cp.tile([128, 384], BF16, tag=f"w2f{k}")
                nc.sync.dma_start(t[:], d_w2f[k])
                w2f.append(t)
            w2hv = []
            for k in range(5):
                t = cp.tile([128, 128], BF16, tag=f"w2hv{k}")
                nc.sync.dma_start(t[:], d_w2hv[k])
                w2hv.append(t)
            w3 = []
            for k in range(8):
                t = cp.tile([128, OF], BF16, tag=f"w3_{k}")
                nc.sync.dma_start(t[:], d_w3[k])
                w3.append(t)

            prev = [None]  # out-step closures of previous tile

            def make_out_steps(n0, A2):
                state = {}

                def step(q, half):
                    b0 = q * 128
                    if half == 0:
                        state[q] = outp.tile([128, OF], F32, tag="outt",
                                             name=f"o_{n0}_{q}")
                    o = state[q]
                    c0 = half * 800
                    ps3 = pp3.tile([128, 800], F32, tag="ps3")
                    for i in range(8):
                        lh = A2[i][:, b0:b0 + 128]
                        st, sp = (i == 0), (i == 7)
                        nc.tensor.matmul(ps3[:, 0:512], lh,
                                         w3[i][:, c0:c0 + 512],
                                         start=st, stop=sp)
                        nc.tensor.matmul(ps3[:, 512:800], lh,
                                         w3[i][:, c0 + 512:c0 + 800],
                                         start=st, stop=sp)
                    nc.scalar.activation(o[:, c0:c0 + 800], ps3[:],
                                         LRELU, alpha=SLOPE)
                    if half == 1:
                        nc.sync.dma_start(d_y[n0 + b0:n0 + b0 + 128, :],
                                          o[:])

                return [lambda q=q, h=h: step(q, h)
                        for q in range(4) for h in range(2)]

            def out_step(k):
                if prev[0] is not None:
                    prev[0][k]()

            for t_i in range(NTILES):
                n0 = t_i * NT

                oh = []
                for c in range(19):
                    t = ohp.tile([128, 2, NT], F8, tag=f"oh{c}",
                                 name=f"oh{c}_{t_i}")
                    nc.sync.dma_start(t[:], d_oh[c, :, :, n0:n0 + NT])
                    oh.append(t)

                A2 = [a2p.tile([128, NT], BF16, tag=f"a2_{i}",
                               name=f"a2_{i}_{t_i}") for i in range(8)]

                act_idx = [0]

                def lrelu_act(dst, ps):
                    act_idx[0] += 1
                    if act_idx[0] > 25 - L1HV_VEC:
                        tmp = vtp.tile([128, NT], F32, tag="vtmp")
                        nc.vector.tensor_scalar(tmp[:], ps[:], SLOPE, None,
                                                op0=mybir.AluOpType.mult)
                        nc.vector.tensor_tensor(dst, tmp[:], ps[:],
                                                op=mybir.AluOpType.max)
                    else:
                        nc.scalar.activation(dst, ps[:], LRELU, alpha=SLOPE)

                # ===== L1 full (fp8 DR, weight hi/lo split) =====
                act1f = []
                for k in range(4):
                    ps = pp.tile([128, NT], F32, tag="ps")
                    nc.tensor.matmul(ps[:], a1f[k][:], oh[k][:],
                                     start=True, stop=True, perf_mode=DRM)
                    a = a1fp.tile([128, NT], BF16, tag=f"act1f{k}",
                                  name=f"act1f{k}_{t_i}")
                    nc.scalar.activation(a[:], ps[:], LRELU, alpha=SLOPE)
                    act1f.append(a)
                out_step(0)

                # ===== L1 h/v per r =====
                stacks = []
                for r in range(5):
                    stk = [stkp.tile([128, NT], BF16, tag=f"stk{r}_{k}",
                                     name=f"stk{r}_{k}_{t_i}")
                           for k in range(5)]
                    ps_hv = []
                    for a1m, ohi in ((a1h, oh[4 + r]), (a1v, oh[9 + r])):
                        for m0 in (0, 128):
                            p = pp.tile([128, NT], F32, tag="ps")
                            nc.tensor.matmul(p[:], a1m[:, :, m0:m0 + 128],
                                             ohi[:], start=True,
                                             stop=True, perf_mode=DRM)
                            ps_hv.append(p)
                    lrelu_act(stk[0][:], ps_hv[0])
                    lrelu_act(stk[1][:], ps_hv[1])
                    lrelu_act(stk[2][:], ps_hv[2])
                    lrelu_act(stk[3][:], ps_hv[3])
                    # tails: one matmul off the 12-row tails one-hot tile
                    ps4 = pp.tile([128, NT], F32, tag="ps")
                    nc.tensor.matmul(ps4[:], a1t[:], oh[14 + r][:],
                                     start=True, stop=True, perf_mode=DRM)
                    lrelu_act(stk[4][:], ps4)
                    stacks.append(stk)
                    out_step(1 + r)

                # ===== L2 full (bf16) =====
                for mc in range(3):
                    m0 = mc * 128
                    ps = pp.tile([128, NT], F32, tag="ps")
                    for k in range(4):
                        nc.tensor.matmul(ps[:], w2f[k][:, m0:m0 + 128],
                                         act1f[k][:],
                                         start=(k == 0), stop=(k == 3))
                    nc.scalar.activation(A2[L2F_DST[mc]][:], ps[:], LRELU,
                                         alpha=SLOPE)
                out_step(6)

                # ===== L2 hv (bf16) =====
                for r in range(5):
                    ps2 = pp.tile([128, NT], F32, tag="ps")
                    for k in range(5):
                        nc.tensor.matmul(ps2[:], w2hv[k][:], stacks[r][k][:],
                                         start=(k == 0), stop=(k == 4))
                    nc.scalar.activation(A2[L2HV_DST[r]][:], ps2[:], LRELU,
                                         alpha=SLOPE)

                out_step(7)
                prev[0] = make_out_steps(n0, A2)

            for k in range(8):
                out_step(k)

    nc.compile()
    return nc


_NC_CACHE = None


def _get_nc():
    global _NC_CACHE
    if _NC_CACHE is None:
        _NC_CACHE = _build_nc()
    return _NC_CACHE


def _prep_weights(inputs):
    f32 = np.float32
    W_df = np.asarray(inputs["W_df"], f32)
    b_df = np.asarray(inputs["b_df"], f32)
    W_pf = np.asarray(inputs["W_pf"], f32)
    b_pf = np.asarray(inputs["b_pf"], f32)
    W_dh = np.asarray(inputs["W_dh"], f32)
    b_dh = np.asarray(inputs["b_dh"], f32)
    W_ph = np.asarray(inputs["W_ph"], f32)
    b_ph = np.asarray(inputs["b_ph"], f32)
    W_dv = np.asarray(inputs["W_dv"], f32)
    b_dv = np.asarray(inputs["b_dv"], f32)
    W_pv = np.asarray(inputs["W_pv"], f32)
    b_pv = np.asarray(inputs["b_pv"], f32)
    W_out = np.asarray(inputs["W_out"], f32)
    b_out = np.asarray(inputs["b_out"], f32)

    # ---- L1 full: per-chunk [in 128, out 128] block-diag matrices ----
    # chunk kc holds classes 5kc..5kc+4 at rows/cols 25*(c%5)+{l,m};
    # ones-in row at ONES_F[kc] carries the bias; chunk3 col 50 emits the
    # act1f ones row used for the L2f bias.
    a1f = np.zeros((4, 128, 2, 128), F8NP)
    ONES_F = [125, 125, 125, 50]
    for kc in range(4):
        A = np.zeros((128, 128), f32)
        for c in range(5 * kc, min(5 * kc + 5, NCL)):
            b0 = 25 * (c % 5)
            A[b0:b0 + 25, b0:b0 + 25] = W_df[c].T  # [l, m]
            A[ONES_F[kc], b0:b0 + 25] = b_df[25 * c:25 * c + 25]
        if kc == 3:
            A[50, 50] = 1.0
        hi, lo = _wsplit16(A)
        a1f[kc, :, 0, :] = hi
        a1f[kc, :, 1, :] = lo

    # ---- L1 h/v: [in 128 (5c+j | 5c+r, ones@85), 2, 256] ----
    # out cols 0..255 = feats of classes 0..15; class 16 + ones go to a1t
    def build_a1hv(Wd, bd):
        A = np.zeros((128, 256), f32)
        for c in range(16):
            for m in range(MULT):
                k = 16 * c + m
                A[5 * c:5 * c + 5, k] = Wd[c, m, :]
                A[85, k] = bd[k]
        hi, lo = _wsplit16(A)
        out = np.zeros((128, 2, 256), F8NP)
        out[:, 0, :] = hi
        out[:, 1, :] = lo
        return out

    a1h = build_a1hv(W_dh, b_dh)
    a1v = build_a1hv(W_dv, b_dv)

    # ---- tails matrix: in rows 0..5 = oh_h[80:86], 6..11 = oh_v[80:86]
    # out = stack c4: cols 0..15 v-tail, 16 v-ones, 64..79 h-tail, 80 h-ones
    At = np.zeros((128, 128), f32)
    for j in range(5):
        At[j, 64:80] = W_dh[16, :, j]
    At[5, 64:80] = b_dh[256:272]
    At[5, 80] = 1.0
    for rr in range(5):
        At[6 + rr, 0:16] = W_dv[16, :, rr]
    At[11, 0:16] = b_dv[256:272]
    At[11, 16] = 1.0
    hi, lo = _wsplit16(At)
    a1t = np.zeros((128, 2, 128), F8NP)
    a1t[:, 0, :] = hi
    a1t[:, 1, :] = lo

    # ---- L2 full: w2f[k] [128 rows = act1f chunk k, 384 cols] ----
    # cols 0..319 = F outputs, col 320 = A2 ones row, cols 321.. = 0
    w2f = np.zeros((4, 128, 384), BF16NP)
    for k in range(4):
        A = np.zeros((128, 384), f32)
        for c in range(5 * k, min(5 * k + 5, NCL)):
            b0 = 25 * (c % 5)
            A[b0:b0 + 25, 0:320] = W_pf[:, 25 * c:25 * c + 25].T
        if k == 3:
            A[50, 0:320] = b_pf
            A[50, 320] = 1.0
        w2f[k] = A.astype(BF16NP)

    # ---- L2 hv: w2hv[k] [128 rows = stack chunk k, 128 = (h64|v64)] ----
    # stack: c0=h[0:128] c1=h[128:256] c2=v[0:128] c3=v[128:256]
    #        c4: 0:16=v[256:272], 16=v-ones, 64:80=h[256:272], 80=h-ones
    w2hv = np.zeros((5, 128, 128), BF16NP)
    for k in range(2):
        A = np.zeros((128, 128), f32)
        A[:, 0:64] = W_ph[:, 128 * k:128 * k + 128].T
        w2hv[k] = A.astype(BF16NP)
        A = np.zeros((128, 128), f32)
        A[:, 64:128] = W_pv[:, 128 * k:128 * k + 128].T
        w2hv[2 + k] = A.astype(BF16NP)
    A = np.zeros((128, 128), f32)
    A[0:16, 64:128] = W_pv[:, 256:272].T
    A[16, 64:128] = b_pv
    A[64:80, 0:64] = W_ph[:, 256:272].T
    A[80, 0:64] = b_ph
    w2hv[4] = A.astype(BF16NP)

    # ---- conv_out W3 chunks [8][128, OF] (A2 row layout) ----
    # A2: c0=F[0:128] c1=F[128:256] c2..c6=HV_r (h 0:64 | v 64:128)
    #     c7: 0:64=F[256:320], 64=ones(b_out), 65..127=0
    W3 = np.zeros((8, 128, OF), f32)
    for p in range(128):
        W3[0, p] = W_out[:, p // 5, p % 5]
        W3[1, p] = W_out[:, (128 + p) // 5, (128 + p) % 5]
    for r in range(5):
        for p in range(64):
            W3[2 + r, p] = W_out[:, p, 5 + r]
            W3[2 + r, 64 + p] = W_out[:, p, 10 + r]
    for p in range(64):
        W3[7, p] = W_out[:, (256 + p) // 5, (256 + p) % 5]
    W3[7, 64] = b_out

    return {
        "a1f": a1f, "a1h": a1h, "a1v": a1v, "a1t": a1t,
        "w2f": w2f, "w2hv": w2hv,
        "w3": W3.astype(BF16NP),
    }


def _build_onehot(x):
    """[14, 128, 2, B] fp8 one-hot (slot0: 1.0, slot1: 1/16) + ones rows.

    idx 0..3: full-branch chunks (class-major); 4+r: h tiles (5c+j);
    9+j: v tiles (5c+r). Ones rows at 125/125/125/50 and 85.
    """
    B = x.shape[0]
    arr = np.zeros((19, 128, 2, B), np.uint8)
    ONE = ONE8.view(np.uint8).item()
    SIXT = SIXT8.view(np.uint8).item()
    bidx = np.arange(B)
    for l in range(25):
        cls = x[:, l]
        kc = cls // 5
        p = 25 * (cls % 5) + l
        arr[kc, p, 0, bidx] = ONE
        arr[kc, p, 1, bidx] = SIXT
        r, j = l // 5, l % 5
        arr[4 + r, 5 * cls + j, 0, bidx] = ONE
        arr[4 + r, 5 * cls + j, 1, bidx] = SIXT
        arr[9 + j, 5 * cls + r, 0, bidx] = ONE
        arr[9 + j, 5 * cls + r, 1, bidx] = SIXT
    for idx, p in [(0, 125), (1, 125), (2, 125), (3, 50)] + \
                  [(4 + i, 85) for i in range(10)]:
        arr[idx, p, 0, :] = ONE
        arr[idx, p, 1, :] = SIXT
    # tails tiles: rows 0..5 = h rows 80..85, rows 6..11 = v rows 80..85
    for r in range(5):
        arr[14 + r, 0:6] = arr[4 + r, 80:86]
        arr[14 + r, 6:12] = arr[9 + r, 80:86]
    return arr.view(F8NP)


def kernel(**inputs) -> np.ndarray:
    x = np.asarray(inputs["x"]).astype(np.int32)
    assert x.shape == (B_FULL, 25), x.shape

    shared = _prep_weights(inputs)
    oh_all = _build_onehot(x)
    nc = _get_nc()

    in_maps = []
    for core in range(NCORES):
        m = dict(shared)
        m["oh"] = np.ascontiguousarray(
            oh_all[:, :, :, core * BC:(core + 1) * BC])
        in_maps.append(m)

    res = run_bass_kernel_spmd(nc, in_maps, core_ids=list(range(NCORES)))
    global LAST_RESULTS
    LAST_RESULTS = res
    out = np.concatenate([res.results[i]["y"] for i in range(NCORES)], axis=0)
    return out


LAST_RESULTS = None
